# revision 39
# baseline (speedup 1.0000x reference)
"""Trainium2 Bass kernel for multi-head attention (B=2, S=2048, D=1024, H=16, HD=64).

Sharding: tensor-parallel over heads. Each of the 8 cores owns 2 heads
(core c -> heads 2c, 2c+1) and computes:
  - q^T, k^T projections for its heads (layout [head_dim*2, seq]); the two
    heads live on partitions 0:64 / 64:128 so their K=64 score matmuls run
    CONCURRENTLY on the PE via row-group tiling
  - v^T projection (N=512 matmuls) + one PE transpose per s-tile into
    [1 | pad63 | v(64)] blocks, so the attended matmul's stationary operand
    carries a ones column: the attended matmul then emits the softmax
    denominators on PSUM partition 0 for free
  - scores^T = K @ Q^T per (batch, head) in [key, query] layout -> exp on
    ScalarE (reading PSUM directly, 2-key-tile groups) with the 1/sqrt(64)
    scale fused into the activation
  - attended^T = [1|pad|V]^T @ exp(scores^T), normalized via
    reciprocal_approx_fast + a GpSimd partition broadcast
  - partial output projection out_c = attended_c @ out_w[:, heads_c]^T
Host sums the 8 partial outputs and adds the bias.

All input transposes are done host-side (only v^T -> v uses the PE's
transpose mode on-chip). Prerequisite projections are hand-woven into the attention emission order
(the Tile schedule is static per engine) and head B's attended matmuls +
normalize + output projection are software-pipelined one iteration later,
under the next iteration's exp calls.
"""

import numpy as np
import ml_dtypes

import concourse.bacc as bacc
import concourse.tile as tile
import concourse.mybir as mybir
from concourse.bass_utils import run_bass_kernel_spmd
from concourse.masks import make_identity

B, S, D = 2, 2048, 1024
H, HD = 16, 64
FEA = H * HD  # 1024
NCORES = 8
BS = B * S  # 4096

DT_TILES = 8      # 1024 contraction dim / 128
JT = 16           # key tiles of 128 per batch
IB = 4            # query blocks of 512 per batch
VW = 256          # v storage width per s-tile: [1|pad63|vA(64) | 1|pad63|vB(64)]

BF16 = mybir.dt.bfloat16
F32 = mybir.dt.float32
AF = mybir.ActivationFunctionType
ALU = mybir.AluOpType

_NC_CACHE = {}


def _emit(tc, xT, wqk, wv, wo, out):
    nc = tc.nc
    with (
        tc.tile_pool(name="consts", bufs=1) as consts,
        tc.tile_pool(name="stp", bufs=3) as stp,
        tc.tile_pool(name="small", bufs=4) as small,
        tc.tile_pool(name="tmpb", bufs=2) as tmpb,
        tc.tile_pool(name="outsb", bufs=3) as outsb,
        tc.tile_pool(name="ps_big", bufs=3, space="PSUM") as ps_big,
        tc.tile_pool(name="ps_att", bufs=1, space="PSUM") as ps_att,
        tc.tile_pool(name="ps_small", bufs=1, space="PSUM") as ps_small,
    ):
        xts = [consts.tile([128, BS], BF16, name=f"xt{i}", tag=f"xt{i}") for i in range(DT_TILES)]
        wqks = [consts.tile([128, 256], BF16, name=f"wqk{i}", tag=f"wqk{i}") for i in range(DT_TILES)]
        wvs = [consts.tile([128, 128], BF16, name=f"wv{i}", tag=f"wv{i}") for i in range(DT_TILES)]
        wos = consts.tile([128, D], BF16, tag="wo")
        # per-batch activations
        qTs = [consts.tile([128, S], BF16, name=f"qT{b}", tag=f"qT{b}") for b in range(B)]
        kTs = [consts.tile([128, S], BF16, name=f"kT{b}", tag=f"kT{b}") for b in range(B)]
        vsbs = [consts.tile([128, 16 * VW], BF16, name=f"v{b}", tag=f"v{b}") for b in range(B)]
        attTs = [consts.tile([128, S], BF16, name=f"attT{b}", tag=f"attT{b}") for b in range(B)]
        vTs = [consts.tile([128, S], BF16, name=f"vT{b}", tag=f"vT{b}") for b in range(B)]
        ident = consts.tile([128, 128], BF16, tag="ident")

        # load x^T in column chunks so QKV (and then attention) can start
        # before the whole 8MB lands; weights ride along with chunk 0
        for cb in range(8):
            for dt in range(DT_TILES):
                if cb == 0:
                    nc.sync.dma_start(out=wqks[dt], in_=wqk[dt * 128:(dt + 1) * 128, :])
                nc.sync.dma_start(
                    out=xts[dt][:, cb * 512:(cb + 1) * 512],
                    in_=xT[dt * 128:(dt + 1) * 128, cb * 512:(cb + 1) * 512],
                )
                if cb == 0:
                    nc.sync.dma_start(out=wvs[dt], in_=wv[dt * 128:(dt + 1) * 128, :])
        nc.sync.dma_start(out=wos, in_=wo[:, :])
        for b in range(B):
            nc.vector.memset(vsbs[b], 1.0)  # presets the ones columns
        make_identity(nc, ident)

        def _emit_proj_nb(b, nb, half, dst):
            # one 512-col block of a q^T / k^T projection
            scol = nb * 512
            ps = ps_small.tile([128, 512], F32, name="pss", tag="ps_small")
            for dt in range(DT_TILES):
                nc.tensor.matmul(
                    ps,
                    lhsT=wqks[dt][:, half * 128:(half + 1) * 128],
                    rhs=xts[dt][:, b * S + scol: b * S + scol + 512],
                    start=(dt == 0),
                    stop=(dt == DT_TILES - 1),
                )
            nc.vector.tensor_copy(out=dst[:, scol:scol + 512], in_=ps)

        def emit_q_nb(b, nb):
            _emit_proj_nb(b, nb, 0, qTs[b])

        def emit_k_nb(b, nb):
            _emit_proj_nb(b, nb, 1, kTs[b])

        def emit_qk_nb(b, nb):
            emit_k_nb(b, nb)
            emit_q_nb(b, nb)

        def emit_vT_nb(b, nb):
            # v^T [f=128, s] projection block: N=512 matmuls with a reused
            # stationary (much cheaper on the PE than 4 tiny N=128 groups)
            scol = nb * 512
            ps = ps_small.tile([128, 512], F32, name="pss", tag="ps_small")
            for dt in range(DT_TILES):
                nc.tensor.matmul(
                    ps,
                    lhsT=wvs[dt],
                    rhs=xts[dt][:, b * S + scol: b * S + scol + 512],
                    start=(dt == 0),
                    stop=(dt == DT_TILES - 1),
                )
            nc.vector.tensor_copy(out=vTs[b][:, scol:scol + 512], in_=ps)

        def emit_v(b, st):
            # One PE transpose turns v^T's [f=128, s-tile] block into natural
            # [s, f] order, then a strided DVE copy lands it as
            # [1|pad63|vA(64) | 1|pad63|vB(64)]. Ones column first =>
            # softmax denominators land on PSUM partition 0
            # (partition_broadcast needs a partition-0 source); attended rows
            # occupy partitions 64:128 (spans >32 partitions must start at 0
            # or 64). Pad columns are 1.0 -> harmless duplicate denominator
            # rows in PSUM.
            ps = ps_small.tile([128, 128], BF16, name="pss", tag="ps_small")
            nc.tensor.transpose(
                ps, vTs[b][:, st * 128:(st + 1) * 128], ident
            )
            vsrc = ps.rearrange("p (two c) -> p two c", two=2)
            vdst = vsbs[b][:, st * VW: st * VW + VW].rearrange(
                "p (two c) -> p two c", two=2
            )[:, :, 64:128]
            nc.vector.tensor_copy(out=vdst, in_=vsrc)

        def emit_outproj_ib(b, ib):
            # output rows [b*S + ib*512, +512) only need attT cols of this ib
            for st in range(b * 16 + ib * 4, b * 16 + ib * 4 + 4):
                for db in range(2):
                    ps = ps_small.tile([128, 512], F32, name="pss", tag="ps_small")
                    nc.tensor.matmul(
                        ps,
                        lhsT=attTs[b][:, (st - b * 16) * 128:(st - b * 16 + 1) * 128],
                        rhs=wos[:, db * 512:(db + 1) * 512],
                        start=True,
                        stop=True,
                    )
                    osb = outsb.tile([128, 512], F32, name="osb", tag="osb")
                    nc.vector.tensor_copy(out=osb, in_=ps)
                    nc.sync.dma_start(
                        out=out[st * 128:(st + 1) * 128, db * 512:(db + 1) * 512],
                        in_=osb,
                    )

        def normalize(b, ib, h, att_ps):
            icol = ib * 512
            # Drain PSUM immediately (one copy) so the slot frees fast;
            # normalize from the SBUF copy. Row 0 = denominators,
            # rows 64:128 = attended^T.
            araw = small.tile([128, 512], F32, name="araw", tag="araw")
            nc.vector.tensor_copy(out=araw, in_=att_ps[0:128, :])
            rrow = small.tile([1, 512], F32, name="rrow", tag="rrow")
            nc.vector.reciprocal_approx_fast(out=rrow, in_=araw[0:1, :])
            rb = small.tile([128, 512], F32, name="rb", tag="rb")
            nc.gpsimd.partition_broadcast(rb, rrow)
            # tensor_tensor needs both SBUF inputs at the same base
            # partition -> use the 64:128 half of the broadcast.
            if h == 0:
                nc.vector.tensor_tensor(
                    out=attTs[b][0:64, icol:icol + 512],
                    in0=araw[64:128, :],
                    in1=rb[64:128, :],
                    op=ALU.mult,
                )
            else:
                tb = tmpb.tile([64, 512], BF16, name="tb", tag="tb")
                nc.vector.tensor_tensor(
                    out=tb,
                    in0=araw[64:128, :],
                    in1=rb[64:128, :],
                    op=ALU.mult,
                )
                # head B lives at partitions 64:128 of attT; engines
                # can't cross partitions, DMA can.
                nc.sync.dma_start(
                    out=attTs[b][64:128, icol:icol + 512], in_=tb
                )

        # (b, ib, slot) -> prerequisite projection blocks to emit there.
        # k blocks must precede the score groups that read them; vT blocks
        # must precede the v transposes of the s-tiles they cover.
        WEAVE = {
            (0, 0, 0): (("qk", (0, 0)), ("vT", (0, 0))),
            (0, 0, 2): (("k", (0, 1)), ("vT", (0, 1))),
            (0, 0, 3): (("q", (0, 1)),),
            (0, 0, 4): (("k", (0, 2)), ("vT", (0, 2))),
            (0, 0, 5): (("q", (0, 2)),),
            (0, 0, 6): (("k", (0, 3)), ("vT", (0, 3))),
            (0, 0, 7): (("q", (0, 3)),),
            (0, 1, 0): (("k", (1, 0)),),
            (0, 1, 1): (("q", (1, 0)),),
            (0, 1, 2): (("k", (1, 1)),),
            (0, 1, 3): (("q", (1, 1)),),
            (0, 1, 4): (("k", (1, 2)),),
            (0, 1, 5): (("q", (1, 2)),),
            (0, 1, 6): (("k", (1, 3)),),
            (0, 1, 7): (("q", (1, 3)),),
            (0, 2, 0): (("vT", (1, 0)),),
            (0, 2, 1): (("vT", (1, 1)),),
            (0, 2, 2): (("vT", (1, 2)), ("v", (1, 0)), ("v", (1, 1))),
            (0, 2, 3): (("vT", (1, 3)), ("v", (1, 2)), ("v", (1, 3))),
            (0, 2, 4): (("v", (1, 4)), ("v", (1, 5))),
            (0, 2, 5): (("v", (1, 6)), ("v", (1, 7))),
            (0, 2, 6): (("v", (1, 8)), ("v", (1, 9))),
            (0, 2, 7): (("v", (1, 10)), ("v", (1, 11))),
            (0, 3, 0): (("v", (1, 12)), ("v", (1, 13))),
            (0, 3, 1): (("v", (1, 14)), ("v", (1, 15))),
        }

        # Head B's attended matmuls + normalize + outproj of iteration (b,ib)
        # are software-pipelined into the START of the next iteration, where
        # the ScalarE stream still lags the PE (so the PE burst hides under
        # the previous iteration's exp calls).
        pend = {}

        def weave_pending(stage):
            if "b" not in pend:
                return
            pb, pib, pstB = pend["b"], pend["ib"], pend["stB"]
            if stage == 0:
                attB_ps = ps_att.tile([128, 512], F32, name="attps", tag="att")
                pend["ps"] = attB_ps
                for jt in range(JT):
                    nc.tensor.matmul(
                        attB_ps[0:128, :],
                        lhsT=vsbs[pb][:, jt * VW + 128: jt * VW + 256],
                        rhs=pstB[:, jt * 512:(jt + 1) * 512],
                        start=(jt == 0),
                        stop=(jt == JT - 1),
                    )
            else:
                normalize(pb, pib, 1, pend["ps"])
                emit_outproj_ib(pb, pib)
                pend.clear()

        def emit_attention_ib(b, ib):
            icol = ib * 512
            # Both heads processed together: head A (partitions 0:64) and
            # head B (64:128) score matmuls are emitted interleaved so the
            # PE runs them CONCURRENTLY via row-group tiling (K=64 each ->
            # disjoint row halves of the array).
            stA = stp.tile([128, JT * 512], BF16, name="stA", tag="st")
            stB = stp.tile([128, JT * 512], BF16, name="stB", tag="st")
            attA_ps = None
            for g0 in range(0, JT, 2):
                g = g0 // 2
                gw = min(2, JT - g0)
                if g == 0:
                    weave_pending(0)
                elif g == 1:
                    weave_pending(1)
                # Weave prerequisite projections into the attention stream
                # so the statically-scheduled PE stream has filler for its
                # idle slices: batch 0/ib 0 pulls in its own k blocks just
                # before the score groups that need them (+ v^T blocks);
                # batch 1's projections ride inside batch 0's later
                # iterations. (qk_half, vT, v_tr) per slot, see table.
                for kind, arg in WEAVE.get((b, ib, g), ()):
                    if kind == "qk":
                        emit_qk_nb(*arg)
                    elif kind == "q":
                        emit_q_nb(*arg)
                    elif kind == "k":
                        emit_k_nb(*arg)
                    elif kind == "v":
                        emit_v(*arg)
                    elif kind == "vT":
                        emit_vT_nb(*arg)
                scA = ps_big.tile([128, 1024], F32, name="scA", tag="sc")
                scB = ps_big.tile([128, 1024], F32, name="scB", tag="sc")
                for idx in range(gw):
                    jt = g0 + idx
                    for hsl, sc in ((slice(0, 64), scA), (slice(64, 128), scB)):
                        nc.tensor.matmul(
                            sc[:, idx * 512:(idx + 1) * 512],
                            lhsT=kTs[b][hsl, jt * 128:(jt + 1) * 128],
                            rhs=qTs[b][hsl, icol:icol + 512],
                            start=True,
                            stop=True,
                        )
                for st_exp, sc in ((stA, scA), (stB, scB)):
                    nc.scalar.activation(
                        out=st_exp[:, g0 * 512:(g0 + gw) * 512],
                        in_=sc[:, 0:gw * 512],
                        func=AF.Exp,
                        scale=0.125,
                    )
                # batch 0's v tiles just in time for the attended matmuls
                # below (b1's were prebuilt in the weave)
                if b == 0 and ib == 0:
                    for idx in range(gw):
                        emit_v(b, g0 + idx)
                # attended for head A consumes the PREVIOUS group's exps
                # (one group of slack so the PE never waits on ScalarE)
                if attA_ps is None:
                    attA_ps = ps_att.tile([128, 512], F32, name="attps", tag="att")
                if g0 >= 2:
                    for jt in (g0 - 2, g0 - 1):
                        nc.tensor.matmul(
                            attA_ps[0:128, :],
                            lhsT=vsbs[b][:, jt * VW: jt * VW + 128],
                            rhs=stA[:, jt * 512:(jt + 1) * 512],
                            start=(jt == 0),
                            stop=False,
                        )

            for jt in (JT - 2, JT - 1):
                nc.tensor.matmul(
                    attA_ps[0:128, :],
                    lhsT=vsbs[b][:, jt * VW: jt * VW + 128],
                    rhs=stA[:, jt * 512:(jt + 1) * 512],
                    start=False,
                    stop=(jt == JT - 1),
                )
            normalize(b, ib, 0, attA_ps)
            pend.update(b=b, ib=ib, stB=stB)

        for b in range(B):
            for ib in range(IB):
                emit_attention_ib(b, ib)
        weave_pending(0)
        weave_pending(1)


def build_nc():
    if "nc" in _NC_CACHE:
        return _NC_CACHE["nc"]
    nc = bacc.Bacc("TRN2", debug=False, num_devices=NCORES)
    xT = nc.dram_tensor("xT", [D, BS], BF16, kind="ExternalInput").ap()
    wqk = nc.dram_tensor("wqk", [D, 256], BF16, kind="ExternalInput").ap()
    wv = nc.dram_tensor("wv", [D, 128], BF16, kind="ExternalInput").ap()
    wo = nc.dram_tensor("wo", [128, D], BF16, kind="ExternalInput").ap()
    out = nc.dram_tensor("out", [BS, D], F32, kind="ExternalOutput").ap()
    with tile.TileContext(nc) as tc:
        _emit(tc, xT, wqk, wv, wo, out)
    nc.compile()
    _NC_CACHE["nc"] = nc
    return nc


def make_in_maps(x, qkv_w):
    """Host-side shard + transpose + cast. Returns per-core input dicts
    (without wo/out, added by caller)."""
    bf = ml_dtypes.bfloat16
    xT = np.ascontiguousarray(x.reshape(BS, D).T).astype(bf)
    maps = []
    for c in range(NCORES):
        wA = qkv_w[c * 384: c * 384 + 192]
        wB = qkv_w[c * 384 + 192: c * 384 + 384]
        wq = np.concatenate([wA[0:64], wB[0:64]], 0)        # [128, D]
        wk = np.concatenate([wA[64:128], wB[64:128]], 0)    # [128, D]
        wv_ = np.concatenate([wA[128:192], wB[128:192]], 0)  # [128, D]
        wqk_c = np.ascontiguousarray(
            np.concatenate([wq, wk], 0).T).astype(bf)        # [D, 256]
        wv_c = np.ascontiguousarray(wv_.T).astype(bf)        # [D, 128]
        maps.append({"xT": xT, "wqk": wqk_c, "wv": wv_c})
    return maps


def kernel(x, qkv_w, out_w, out_b, _run_kwargs=None):
    x = np.asarray(x, dtype=np.float32)
    qkv_w = np.asarray(qkv_w, dtype=np.float32)
    out_w = np.asarray(out_w, dtype=np.float32)
    out_b = np.asarray(out_b, dtype=np.float32)
    bf = ml_dtypes.bfloat16

    nc = build_nc()
    in_maps = make_in_maps(x, qkv_w)
    for c in range(NCORES):
        wo_c = np.ascontiguousarray(
            out_w[:, c * 128:(c + 1) * 128].T).astype(bf)    # [128, D]
        in_maps[c]["wo"] = wo_c

    res = run_bass_kernel_spmd(
        nc, in_maps, list(range(NCORES)), **(_run_kwargs or {})
    )
    total = np.zeros((BS, D), np.float32)
    for c in range(NCORES):
        total += np.asarray(res.results[c]["out"])
    total += out_b[None, :]
    out = total.reshape(B, S, D)
    if _run_kwargs:
        kernel.last_result = res
    return out


# revision 40
# speedup vs baseline: 1.0617x; 1.0617x over previous
"""Trainium2 Bass kernel for multi-head attention (B=2, S=2048, D=1024, H=16, HD=64).

Sharding: tensor-parallel over heads. Each of the 8 cores owns 2 heads
(core c -> heads 2c, 2c+1) and computes:
  - q^T, k^T projections for its heads (layout [head_dim*2, seq]); the two
    heads live on partitions 0:64 / 64:128 so their K=64 score matmuls run
    CONCURRENTLY on the PE via row-group tiling
  - v^T projection (N=512 matmuls) + one PE transpose per s-tile into
    [1 | pad63 | v(64)] blocks, so the attended matmul's stationary operand
    carries a ones column: the attended matmul then emits the softmax
    denominators on PSUM partition 0 for free
  - scores^T = K @ Q^T per (batch, head) in [key, query] layout -> exp on
    ScalarE (reading PSUM directly, 2-key-tile groups) with the 1/sqrt(64)
    scale fused into the activation
  - attended^T = [1|pad|V]^T @ exp(scores^T), normalized via
    reciprocal_approx_fast + a GpSimd partition broadcast
  - partial output projection out_c = attended_c @ out_w[:, heads_c]^T
Host sums the 8 partial outputs and adds the bias.

All input transposes are done host-side (only v^T -> v uses the PE's
transpose mode on-chip). Prerequisite projections are hand-woven into the attention emission order
(the Tile schedule is static per engine) and head B's attended matmuls +
normalize + output projection are software-pipelined one iteration later,
under the next iteration's exp calls.
"""

import numpy as np
import ml_dtypes

import concourse.bacc as bacc
import concourse.tile as tile
import concourse.mybir as mybir
from concourse.bass_utils import run_bass_kernel_spmd
from concourse.masks import make_identity

B, S, D = 2, 2048, 1024
H, HD = 16, 64
FEA = H * HD  # 1024
NCORES = 8
BS = B * S  # 4096

DT_TILES = 8      # 1024 contraction dim / 128
JT = 16           # key tiles of 128 per batch
IB = 4            # query blocks of 512 per batch
VW = 256          # v storage width per s-tile: [1|pad63|vA(64) | 1|pad63|vB(64)]

BF16 = mybir.dt.bfloat16
F32 = mybir.dt.float32
AF = mybir.ActivationFunctionType
ALU = mybir.AluOpType

_NC_CACHE = {}


def _emit(tc, xT, wqk, wv, wo, out):
    nc = tc.nc
    with (
        tc.tile_pool(name="consts", bufs=1) as consts,
        tc.tile_pool(name="stp", bufs=3) as stp,
        tc.tile_pool(name="small", bufs=4) as small,
        tc.tile_pool(name="tmpb", bufs=2) as tmpb,
        tc.tile_pool(name="outsb", bufs=3) as outsb,
        tc.tile_pool(name="ps_big", bufs=2, space="PSUM") as ps_big,
        tc.tile_pool(name="ps_att", bufs=2, space="PSUM") as ps_att,
        tc.tile_pool(name="ps_small", bufs=2, space="PSUM") as ps_small,
    ):
        xts = [consts.tile([128, BS], BF16, name=f"xt{i}", tag=f"xt{i}") for i in range(DT_TILES)]
        wqks = [consts.tile([128, 256], BF16, name=f"wqk{i}", tag=f"wqk{i}") for i in range(DT_TILES)]
        wvs = [consts.tile([128, 128], BF16, name=f"wv{i}", tag=f"wv{i}") for i in range(DT_TILES)]
        wos = consts.tile([128, D], BF16, tag="wo")
        # per-batch activations
        qTs = [consts.tile([128, S], BF16, name=f"qT{b}", tag=f"qT{b}") for b in range(B)]
        kTs = [consts.tile([128, S], BF16, name=f"kT{b}", tag=f"kT{b}") for b in range(B)]
        vsbs = [consts.tile([128, 16 * VW], BF16, name=f"v{b}", tag=f"v{b}") for b in range(B)]
        attTs = [consts.tile([128, S], BF16, name=f"attT{b}", tag=f"attT{b}") for b in range(B)]
        vTs = [consts.tile([128, S], BF16, name=f"vT{b}", tag=f"vT{b}") for b in range(B)]
        ident = consts.tile([128, 128], BF16, tag="ident")

        # load x^T in column chunks so QKV (and then attention) can start
        # before the whole 8MB lands; weights ride along with chunk 0
        for cb in range(8):
            for dt in range(DT_TILES):
                if cb == 0:
                    nc.sync.dma_start(out=wqks[dt], in_=wqk[dt * 128:(dt + 1) * 128, :])
                nc.sync.dma_start(
                    out=xts[dt][:, cb * 512:(cb + 1) * 512],
                    in_=xT[dt * 128:(dt + 1) * 128, cb * 512:(cb + 1) * 512],
                )
                if cb == 0:
                    nc.sync.dma_start(out=wvs[dt], in_=wv[dt * 128:(dt + 1) * 128, :])
        nc.sync.dma_start(out=wos, in_=wo[:, :])
        for b in range(B):
            nc.vector.memset(vsbs[b], 1.0)  # presets the ones columns
        make_identity(nc, ident)

        def _emit_proj_nb(b, nb, half, dst):
            # one 512-col block of a q^T / k^T projection
            scol = nb * 512
            ps = ps_small.tile([128, 512], F32, name="pss", tag="ps_small")
            for dt in range(DT_TILES):
                nc.tensor.matmul(
                    ps,
                    lhsT=wqks[dt][:, half * 128:(half + 1) * 128],
                    rhs=xts[dt][:, b * S + scol: b * S + scol + 512],
                    start=(dt == 0),
                    stop=(dt == DT_TILES - 1),
                )
            nc.vector.tensor_copy(out=dst[:, scol:scol + 512], in_=ps)

        def emit_q_nb(b, nb):
            _emit_proj_nb(b, nb, 0, qTs[b])

        def emit_k_nb(b, nb):
            _emit_proj_nb(b, nb, 1, kTs[b])

        def emit_qk_nb(b, nb):
            emit_k_nb(b, nb)
            emit_q_nb(b, nb)

        def emit_vT_nb(b, nb):
            # v^T [f=128, s] projection block: N=512 matmuls with a reused
            # stationary (much cheaper on the PE than 4 tiny N=128 groups)
            scol = nb * 512
            ps = ps_small.tile([128, 512], F32, name="pss", tag="ps_small")
            for dt in range(DT_TILES):
                nc.tensor.matmul(
                    ps,
                    lhsT=wvs[dt],
                    rhs=xts[dt][:, b * S + scol: b * S + scol + 512],
                    start=(dt == 0),
                    stop=(dt == DT_TILES - 1),
                )
            nc.vector.tensor_copy(out=vTs[b][:, scol:scol + 512], in_=ps)

        def emit_v(b, st):
            # One PE transpose turns v^T's [f=128, s-tile] block into natural
            # [s, f] order, then a strided DVE copy lands it as
            # [1|pad63|vA(64) | 1|pad63|vB(64)]. Ones column first =>
            # softmax denominators land on PSUM partition 0
            # (partition_broadcast needs a partition-0 source); attended rows
            # occupy partitions 64:128 (spans >32 partitions must start at 0
            # or 64). Pad columns are 1.0 -> harmless duplicate denominator
            # rows in PSUM.
            ps = ps_small.tile([128, 128], BF16, name="pss", tag="ps_small")
            nc.tensor.transpose(
                ps, vTs[b][:, st * 128:(st + 1) * 128], ident
            )
            vsrc = ps.rearrange("p (two c) -> p two c", two=2)
            vdst = vsbs[b][:, st * VW: st * VW + VW].rearrange(
                "p (two c) -> p two c", two=2
            )[:, :, 64:128]
            nc.vector.tensor_copy(out=vdst, in_=vsrc)

        def emit_outproj_ib(b, ib):
            # output rows [b*S + ib*512, +512) only need attT cols of this ib
            for st in range(b * 16 + ib * 4, b * 16 + ib * 4 + 4):
                for db in range(2):
                    ps = ps_small.tile([128, 512], F32, name="pss", tag="ps_small")
                    nc.tensor.matmul(
                        ps,
                        lhsT=attTs[b][:, (st - b * 16) * 128:(st - b * 16 + 1) * 128],
                        rhs=wos[:, db * 512:(db + 1) * 512],
                        start=True,
                        stop=True,
                    )
                    osb = outsb.tile([128, 512], F32, name="osb", tag="osb")
                    nc.vector.tensor_copy(out=osb, in_=ps)
                    nc.sync.dma_start(
                        out=out[st * 128:(st + 1) * 128, db * 512:(db + 1) * 512],
                        in_=osb,
                    )

        def normalize(b, ib, h, att_ps):
            icol = ib * 512
            # Drain PSUM immediately (one copy) so the slot frees fast;
            # normalize from the SBUF copy. Row 0 = denominators,
            # rows 64:128 = attended^T.
            araw = small.tile([128, 512], F32, name="araw", tag="araw")
            nc.vector.tensor_copy(out=araw, in_=att_ps[0:128, :])
            rrow = small.tile([1, 512], F32, name="rrow", tag="rrow")
            nc.vector.reciprocal_approx_fast(out=rrow, in_=araw[0:1, :])
            rb = small.tile([128, 512], F32, name="rb", tag="rb")
            nc.gpsimd.partition_broadcast(rb, rrow)
            # tensor_tensor needs both SBUF inputs at the same base
            # partition -> use the 64:128 half of the broadcast.
            if h == 0:
                nc.vector.tensor_tensor(
                    out=attTs[b][0:64, icol:icol + 512],
                    in0=araw[64:128, :],
                    in1=rb[64:128, :],
                    op=ALU.mult,
                )
            else:
                tb = tmpb.tile([64, 512], BF16, name="tb", tag="tb")
                nc.vector.tensor_tensor(
                    out=tb,
                    in0=araw[64:128, :],
                    in1=rb[64:128, :],
                    op=ALU.mult,
                )
                # head B lives at partitions 64:128 of attT; engines
                # can't cross partitions, DMA can.
                nc.sync.dma_start(
                    out=attTs[b][64:128, icol:icol + 512], in_=tb
                )

        # (b, ib, slot) -> prerequisite projection blocks to emit there.
        # k blocks must precede the score groups that read them; vT blocks
        # must precede the v transposes of the s-tiles they cover.
        WEAVE = {
            (0, 0, 0): (("qk", (0, 0)), ("vT", (0, 0))),
            (0, 0, 2): (("k", (0, 1)), ("vT", (0, 1))),
            (0, 0, 3): (("q", (0, 1)),),
            (0, 0, 4): (("k", (0, 2)), ("vT", (0, 2))),
            (0, 0, 5): (("q", (0, 2)),),
            (0, 0, 6): (("k", (0, 3)), ("vT", (0, 3))),
            (0, 0, 7): (("q", (0, 3)),),
            (0, 1, 0): (("k", (1, 0)),),
            (0, 1, 1): (("q", (1, 0)),),
            (0, 1, 2): (("k", (1, 1)),),
            (0, 1, 3): (("q", (1, 1)),),
            (0, 1, 4): (("k", (1, 2)),),
            (0, 1, 5): (("q", (1, 2)),),
            (0, 1, 6): (("k", (1, 3)),),
            (0, 1, 7): (("q", (1, 3)),),
            (0, 2, 0): (("vT", (1, 0)),),
            (0, 2, 1): (("vT", (1, 1)),),
            (0, 2, 2): (("vT", (1, 2)), ("v", (1, 0)), ("v", (1, 1))),
            (0, 2, 3): (("vT", (1, 3)), ("v", (1, 2)), ("v", (1, 3))),
            (0, 2, 4): (("v", (1, 4)), ("v", (1, 5))),
            (0, 2, 5): (("v", (1, 6)), ("v", (1, 7))),
            (0, 2, 6): (("v", (1, 8)), ("v", (1, 9))),
            (0, 2, 7): (("v", (1, 10)), ("v", (1, 11))),
            (0, 3, 0): (("v", (1, 12)), ("v", (1, 13))),
            (0, 3, 1): (("v", (1, 14)), ("v", (1, 15))),
        }

        # Head B's attended matmuls + normalize + outproj of iteration (b,ib)
        # are software-pipelined into the START of the next iteration, where
        # the ScalarE stream still lags the PE (so the PE burst hides under
        # the previous iteration's exp calls).
        pend = {}

        def weave_pending(stage):
            if "b" not in pend:
                return
            pb, pib, pstB = pend["b"], pend["ib"], pend["stB"]
            if stage == 0:
                attB_ps = ps_att.tile([128, 512], F32, name="attps", tag="att")
                pend["ps"] = attB_ps
                for jt in range(JT):
                    nc.tensor.matmul(
                        attB_ps[0:128, :],
                        lhsT=vsbs[pb][:, jt * VW + 128: jt * VW + 256],
                        rhs=pstB[:, jt * 512:(jt + 1) * 512],
                        start=(jt == 0),
                        stop=(jt == JT - 1),
                    )
            else:
                normalize(pb, pib, 1, pend["ps"])
                emit_outproj_ib(pb, pib)
                pend.clear()

        def emit_attention_ib(b, ib):
            icol = ib * 512
            # Both heads processed together: head A (partitions 0:64) and
            # head B (64:128) score matmuls are emitted interleaved so the
            # PE runs them CONCURRENTLY via row-group tiling (K=64 each ->
            # disjoint row halves of the array).
            stA = stp.tile([128, JT * 512], BF16, name="stA", tag="st")
            stB = stp.tile([128, JT * 512], BF16, name="stB", tag="st")
            attA_ps = None
            for g0 in range(0, JT, 2):
                g = g0 // 2
                gw = min(2, JT - g0)
                if g == 0:
                    weave_pending(0)
                elif g == 1:
                    weave_pending(1)
                # Weave prerequisite projections into the attention stream
                # so the statically-scheduled PE stream has filler for its
                # idle slices: batch 0/ib 0 pulls in its own k blocks just
                # before the score groups that need them (+ v^T blocks);
                # batch 1's projections ride inside batch 0's later
                # iterations. (qk_half, vT, v_tr) per slot, see table.
                for kind, arg in WEAVE.get((b, ib, g), ()):
                    if kind == "qk":
                        emit_qk_nb(*arg)
                    elif kind == "q":
                        emit_q_nb(*arg)
                    elif kind == "k":
                        emit_k_nb(*arg)
                    elif kind == "v":
                        emit_v(*arg)
                    elif kind == "vT":
                        emit_vT_nb(*arg)
                scA = ps_big.tile([128, 1024], F32, name="scA", tag="sc")
                scB = ps_big.tile([128, 1024], F32, name="scB", tag="sc")
                for idx in range(gw):
                    jt = g0 + idx
                    for hsl, sc in ((slice(0, 64), scA), (slice(64, 128), scB)):
                        nc.tensor.matmul(
                            sc[:, idx * 512:(idx + 1) * 512],
                            lhsT=kTs[b][hsl, jt * 128:(jt + 1) * 128],
                            rhs=qTs[b][hsl, icol:icol + 512],
                            start=True,
                            stop=True,
                        )
                for st_exp, sc in ((stA, scA), (stB, scB)):
                    nc.scalar.activation(
                        out=st_exp[:, g0 * 512:(g0 + gw) * 512],
                        in_=sc[:, 0:gw * 512],
                        func=AF.Exp,
                        scale=0.125,
                    )
                # batch 0's v tiles just in time for the attended matmuls
                # below (b1's were prebuilt in the weave)
                if b == 0 and ib == 0:
                    for idx in range(gw):
                        emit_v(b, g0 + idx)
                # attended for head A consumes the PREVIOUS group's exps
                # (one group of slack so the PE never waits on ScalarE)
                if attA_ps is None:
                    attA_ps = ps_att.tile([128, 512], F32, name="attps", tag="att")
                if g0 >= 2:
                    for jt in (g0 - 2, g0 - 1):
                        nc.tensor.matmul(
                            attA_ps[0:128, :],
                            lhsT=vsbs[b][:, jt * VW: jt * VW + 128],
                            rhs=stA[:, jt * 512:(jt + 1) * 512],
                            start=(jt == 0),
                            stop=False,
                        )

            for jt in (JT - 2, JT - 1):
                nc.tensor.matmul(
                    attA_ps[0:128, :],
                    lhsT=vsbs[b][:, jt * VW: jt * VW + 128],
                    rhs=stA[:, jt * 512:(jt + 1) * 512],
                    start=False,
                    stop=(jt == JT - 1),
                )
            normalize(b, ib, 0, attA_ps)
            pend.update(b=b, ib=ib, stB=stB)

        for b in range(B):
            for ib in range(IB):
                emit_attention_ib(b, ib)
        weave_pending(0)
        weave_pending(1)


def build_nc():
    if "nc" in _NC_CACHE:
        return _NC_CACHE["nc"]
    nc = bacc.Bacc("TRN2", debug=False, num_devices=NCORES)
    xT = nc.dram_tensor("xT", [D, BS], BF16, kind="ExternalInput").ap()
    wqk = nc.dram_tensor("wqk", [D, 256], BF16, kind="ExternalInput").ap()
    wv = nc.dram_tensor("wv", [D, 128], BF16, kind="ExternalInput").ap()
    wo = nc.dram_tensor("wo", [128, D], BF16, kind="ExternalInput").ap()
    out = nc.dram_tensor("out", [BS, D], F32, kind="ExternalOutput").ap()
    with tile.TileContext(nc) as tc:
        _emit(tc, xT, wqk, wv, wo, out)
    nc.compile()
    _NC_CACHE["nc"] = nc
    return nc


def make_in_maps(x, qkv_w):
    """Host-side shard + transpose + cast. Returns per-core input dicts
    (without wo/out, added by caller)."""
    bf = ml_dtypes.bfloat16
    xT = np.ascontiguousarray(x.reshape(BS, D).T).astype(bf)
    maps = []
    for c in range(NCORES):
        wA = qkv_w[c * 384: c * 384 + 192]
        wB = qkv_w[c * 384 + 192: c * 384 + 384]
        wq = np.concatenate([wA[0:64], wB[0:64]], 0)        # [128, D]
        wk = np.concatenate([wA[64:128], wB[64:128]], 0)    # [128, D]
        wv_ = np.concatenate([wA[128:192], wB[128:192]], 0)  # [128, D]
        wqk_c = np.ascontiguousarray(
            np.concatenate([wq, wk], 0).T).astype(bf)        # [D, 256]
        wv_c = np.ascontiguousarray(wv_.T).astype(bf)        # [D, 128]
        maps.append({"xT": xT, "wqk": wqk_c, "wv": wv_c})
    return maps


def kernel(x, qkv_w, out_w, out_b, _run_kwargs=None):
    x = np.asarray(x, dtype=np.float32)
    qkv_w = np.asarray(qkv_w, dtype=np.float32)
    out_w = np.asarray(out_w, dtype=np.float32)
    out_b = np.asarray(out_b, dtype=np.float32)
    bf = ml_dtypes.bfloat16

    nc = build_nc()
    in_maps = make_in_maps(x, qkv_w)
    for c in range(NCORES):
        wo_c = np.ascontiguousarray(
            out_w[:, c * 128:(c + 1) * 128].T).astype(bf)    # [128, D]
        in_maps[c]["wo"] = wo_c

    res = run_bass_kernel_spmd(
        nc, in_maps, list(range(NCORES)), **(_run_kwargs or {})
    )
    total = np.zeros((BS, D), np.float32)
    for c in range(NCORES):
        total += np.asarray(res.results[c]["out"])
    total += out_b[None, :]
    out = total.reshape(B, S, D)
    if _run_kwargs:
        kernel.last_result = res
    return out


# revision 42
# speedup vs baseline: 1.0718x; 1.0095x over previous
"""Trainium2 Bass kernel for multi-head attention (B=2, S=2048, D=1024, H=16, HD=64).

Sharding: tensor-parallel over heads. Each of the 8 cores owns 2 heads
(core c -> heads 2c, 2c+1) and computes:
  - q^T, k^T projections for its heads (layout [head_dim*2, seq]); the two
    heads live on partitions 0:64 / 64:128 so their K=64 score matmuls run
    CONCURRENTLY on the PE via row-group tiling
  - v^T projection (N=512 matmuls) + one PE transpose per s-tile into
    [1 | pad63 | v(64)] blocks, so the attended matmul's stationary operand
    carries a ones column: the attended matmul then emits the softmax
    denominators on PSUM partition 0 for free
  - scores^T = K @ Q^T per (batch, head) in [key, query] layout -> exp on
    ScalarE (reading PSUM directly, 2-key-tile groups) with the 1/sqrt(64)
    scale fused into the activation
  - attended^T = [1|pad|V]^T @ exp(scores^T), normalized via
    reciprocal_approx_fast + a GpSimd partition broadcast
  - partial output projection out_c = attended_c @ out_w[:, heads_c]^T
Host sums the 8 partial outputs and adds the bias.

All input transposes are done host-side (only v^T -> v uses the PE's
transpose mode on-chip). Prerequisite projections are hand-woven into the attention emission order
(the Tile schedule is static per engine) and head B's attended matmuls +
normalize + output projection are software-pipelined one iteration later,
under the next iteration's exp calls.
"""

import numpy as np
import ml_dtypes

import concourse.bacc as bacc
import concourse.tile as tile
import concourse.mybir as mybir
from concourse.bass_utils import run_bass_kernel_spmd
from concourse.masks import make_identity

B, S, D = 2, 2048, 1024
H, HD = 16, 64
FEA = H * HD  # 1024
NCORES = 8
BS = B * S  # 4096

DT_TILES = 8      # 1024 contraction dim / 128
JT = 16           # key tiles of 128 per batch
IB = 4            # query blocks of 512 per batch
VW = 256          # v storage width per s-tile: [1|pad63|vA(64) | 1|pad63|vB(64)]

BF16 = mybir.dt.bfloat16
F32 = mybir.dt.float32
AF = mybir.ActivationFunctionType
ALU = mybir.AluOpType

_NC_CACHE = {}


def _emit(tc, xT, wqk, wv, wo, out):
    nc = tc.nc
    with (
        tc.tile_pool(name="consts", bufs=1) as consts,
        tc.tile_pool(name="stp", bufs=3) as stp,
        tc.tile_pool(name="small", bufs=4) as small,
        tc.tile_pool(name="tmpb", bufs=3) as tmpb,
        tc.tile_pool(name="outsb", bufs=4) as outsb,
        tc.tile_pool(name="ps_big", bufs=2, space="PSUM") as ps_big,
        tc.tile_pool(name="ps_att", bufs=2, space="PSUM") as ps_att,
        tc.tile_pool(name="ps_small", bufs=2, space="PSUM") as ps_small,
    ):
        xts = [consts.tile([128, BS], BF16, name=f"xt{i}", tag=f"xt{i}") for i in range(DT_TILES)]
        wqks = [consts.tile([128, 256], BF16, name=f"wqk{i}", tag=f"wqk{i}") for i in range(DT_TILES)]
        wvs = [consts.tile([128, 128], BF16, name=f"wv{i}", tag=f"wv{i}") for i in range(DT_TILES)]
        wos = consts.tile([128, D], BF16, tag="wo")
        # per-batch activations
        qTs = [consts.tile([128, S], BF16, name=f"qT{b}", tag=f"qT{b}") for b in range(B)]
        kTs = [consts.tile([128, S], BF16, name=f"kT{b}", tag=f"kT{b}") for b in range(B)]
        vsbs = [consts.tile([128, 16 * VW], BF16, name=f"v{b}", tag=f"v{b}") for b in range(B)]
        attTs = [consts.tile([128, S], BF16, name=f"attT{b}", tag=f"attT{b}") for b in range(B)]
        vTs = [consts.tile([128, S], BF16, name=f"vT{b}", tag=f"vT{b}") for b in range(B)]
        ident = consts.tile([128, 128], BF16, tag="ident")

        # load x^T in column chunks so QKV (and then attention) can start
        # before the whole 8MB lands; weights ride along with chunk 0
        for cb in range(8):
            for dt in range(DT_TILES):
                if cb == 0:
                    nc.sync.dma_start(out=wqks[dt], in_=wqk[dt * 128:(dt + 1) * 128, :])
                nc.sync.dma_start(
                    out=xts[dt][:, cb * 512:(cb + 1) * 512],
                    in_=xT[dt * 128:(dt + 1) * 128, cb * 512:(cb + 1) * 512],
                )
                if cb == 0:
                    nc.sync.dma_start(out=wvs[dt], in_=wv[dt * 128:(dt + 1) * 128, :])
        nc.sync.dma_start(out=wos, in_=wo[:, :])
        for b in range(B):
            nc.vector.memset(vsbs[b], 1.0)  # presets the ones columns
        make_identity(nc, ident)

        def _emit_proj_nb(b, nb, half, dst):
            # one 512-col block of a q^T / k^T projection
            scol = nb * 512
            ps = ps_small.tile([128, 512], F32, name="pss", tag="ps_small")
            for dt in range(DT_TILES):
                nc.tensor.matmul(
                    ps,
                    lhsT=wqks[dt][:, half * 128:(half + 1) * 128],
                    rhs=xts[dt][:, b * S + scol: b * S + scol + 512],
                    start=(dt == 0),
                    stop=(dt == DT_TILES - 1),
                )
            nc.vector.tensor_copy(out=dst[:, scol:scol + 512], in_=ps)

        def emit_q_nb(b, nb):
            _emit_proj_nb(b, nb, 0, qTs[b])

        def emit_k_nb(b, nb):
            _emit_proj_nb(b, nb, 1, kTs[b])

        def emit_qk_nb(b, nb):
            emit_k_nb(b, nb)
            emit_q_nb(b, nb)

        def emit_vT_nb(b, nb):
            # v^T [f=128, s] projection block: N=512 matmuls with a reused
            # stationary (much cheaper on the PE than 4 tiny N=128 groups)
            scol = nb * 512
            ps = ps_small.tile([128, 512], F32, name="pss", tag="ps_small")
            for dt in range(DT_TILES):
                nc.tensor.matmul(
                    ps,
                    lhsT=wvs[dt],
                    rhs=xts[dt][:, b * S + scol: b * S + scol + 512],
                    start=(dt == 0),
                    stop=(dt == DT_TILES - 1),
                )
            nc.vector.tensor_copy(out=vTs[b][:, scol:scol + 512], in_=ps)

        def emit_v(b, st):
            # One PE transpose turns v^T's [f=128, s-tile] block into natural
            # [s, f] order, then a strided DVE copy lands it as
            # [1|pad63|vA(64) | 1|pad63|vB(64)]. Ones column first =>
            # softmax denominators land on PSUM partition 0
            # (partition_broadcast needs a partition-0 source); attended rows
            # occupy partitions 64:128 (spans >32 partitions must start at 0
            # or 64). Pad columns are 1.0 -> harmless duplicate denominator
            # rows in PSUM.
            ps = ps_small.tile([128, 128], BF16, name="pss", tag="ps_small")
            nc.tensor.transpose(
                ps, vTs[b][:, st * 128:(st + 1) * 128], ident
            )
            vsrc = ps.rearrange("p (two c) -> p two c", two=2)
            vdst = vsbs[b][:, st * VW: st * VW + VW].rearrange(
                "p (two c) -> p two c", two=2
            )[:, :, 64:128]
            nc.vector.tensor_copy(out=vdst, in_=vsrc)

        def emit_outproj_ib(b, ib):
            # output rows [b*S + ib*512, +512) only need attT cols of this ib
            for st in range(b * 16 + ib * 4, b * 16 + ib * 4 + 4):
                for db in range(2):
                    ps = ps_small.tile([128, 512], F32, name="pss", tag="ps_small")
                    nc.tensor.matmul(
                        ps,
                        lhsT=attTs[b][:, (st - b * 16) * 128:(st - b * 16 + 1) * 128],
                        rhs=wos[:, db * 512:(db + 1) * 512],
                        start=True,
                        stop=True,
                    )
                    osb = outsb.tile([128, 512], F32, name="osb", tag="osb")
                    nc.vector.tensor_copy(out=osb, in_=ps)
                    nc.sync.dma_start(
                        out=out[st * 128:(st + 1) * 128, db * 512:(db + 1) * 512],
                        in_=osb,
                    )

        def normalize(b, ib, h, att_ps):
            icol = ib * 512
            # Drain PSUM immediately (one copy) so the slot frees fast;
            # normalize from the SBUF copy. Row 0 = denominators,
            # rows 64:128 = attended^T.
            araw = small.tile([128, 512], F32, name="araw", tag="araw")
            nc.vector.tensor_copy(out=araw, in_=att_ps[0:128, :])
            rrow = small.tile([1, 512], F32, name="rrow", tag="rrow")
            nc.vector.reciprocal_approx_fast(out=rrow, in_=araw[0:1, :])
            rb = small.tile([128, 512], F32, name="rb", tag="rb")
            nc.gpsimd.partition_broadcast(rb, rrow)
            # tensor_tensor needs both SBUF inputs at the same base
            # partition -> use the 64:128 half of the broadcast.
            if h == 0:
                nc.vector.tensor_tensor(
                    out=attTs[b][0:64, icol:icol + 512],
                    in0=araw[64:128, :],
                    in1=rb[64:128, :],
                    op=ALU.mult,
                )
            else:
                tb = tmpb.tile([64, 512], BF16, name="tb", tag="tb")
                nc.vector.tensor_tensor(
                    out=tb,
                    in0=araw[64:128, :],
                    in1=rb[64:128, :],
                    op=ALU.mult,
                )
                # head B lives at partitions 64:128 of attT; engines
                # can't cross partitions, DMA can.
                nc.sync.dma_start(
                    out=attTs[b][64:128, icol:icol + 512], in_=tb
                )

        # (b, ib, slot) -> prerequisite projection blocks to emit there.
        # k blocks must precede the score groups that read them; vT blocks
        # must precede the v transposes of the s-tiles they cover.
        WEAVE = {
            (0, 0, 0): (("qk", (0, 0)), ("vT", (0, 0))),
            (0, 0, 2): (("k", (0, 1)), ("vT", (0, 1))),
            (0, 0, 3): (("q", (0, 1)),),
            (0, 0, 4): (("k", (0, 2)), ("vT", (0, 2))),
            (0, 0, 5): (("q", (0, 2)),),
            (0, 0, 6): (("k", (0, 3)), ("vT", (0, 3))),
            (0, 0, 7): (("q", (0, 3)),),
            (0, 1, 0): (("k", (1, 0)),),
            (0, 1, 1): (("q", (1, 0)),),
            (0, 1, 2): (("k", (1, 1)),),
            (0, 1, 3): (("q", (1, 1)),),
            (0, 1, 4): (("k", (1, 2)),),
            (0, 1, 5): (("q", (1, 2)),),
            (0, 1, 6): (("k", (1, 3)),),
            (0, 1, 7): (("q", (1, 3)),),
            (0, 2, 0): (("vT", (1, 0)),),
            (0, 2, 1): (("vT", (1, 1)),),
            (0, 2, 2): (("vT", (1, 2)), ("v", (1, 0)), ("v", (1, 1))),
            (0, 2, 3): (("vT", (1, 3)), ("v", (1, 2)), ("v", (1, 3))),
            (0, 2, 4): (("v", (1, 4)), ("v", (1, 5))),
            (0, 2, 5): (("v", (1, 6)), ("v", (1, 7))),
            (0, 2, 6): (("v", (1, 8)), ("v", (1, 9))),
            (0, 2, 7): (("v", (1, 10)), ("v", (1, 11))),
            (0, 3, 0): (("v", (1, 12)), ("v", (1, 13))),
            (0, 3, 1): (("v", (1, 14)), ("v", (1, 15))),
        }

        # Head B's attended matmuls + normalize + outproj of iteration (b,ib)
        # are software-pipelined into the START of the next iteration, where
        # the ScalarE stream still lags the PE (so the PE burst hides under
        # the previous iteration's exp calls).
        pend = {}

        def weave_pending(stage):
            if "b" not in pend:
                return
            pb, pib, pstB = pend["b"], pend["ib"], pend["stB"]
            if stage == 0:
                attB_ps = ps_att.tile([128, 512], F32, name="attps", tag="att")
                pend["ps"] = attB_ps
                for jt in range(JT):
                    nc.tensor.matmul(
                        attB_ps[0:128, :],
                        lhsT=vsbs[pb][:, jt * VW + 128: jt * VW + 256],
                        rhs=pstB[:, jt * 512:(jt + 1) * 512],
                        start=(jt == 0),
                        stop=(jt == JT - 1),
                    )
            else:
                normalize(pb, pib, 1, pend["ps"])
                emit_outproj_ib(pb, pib)
                pend.clear()

        def emit_attention_ib(b, ib):
            icol = ib * 512
            # Both heads processed together: head A (partitions 0:64) and
            # head B (64:128) score matmuls are emitted interleaved so the
            # PE runs them CONCURRENTLY via row-group tiling (K=64 each ->
            # disjoint row halves of the array).
            stA = stp.tile([128, JT * 512], BF16, name="stA", tag="st")
            stB = stp.tile([128, JT * 512], BF16, name="stB", tag="st")
            attA_ps = None
            for g0 in range(0, JT, 2):
                g = g0 // 2
                gw = min(2, JT - g0)
                if g == 0:
                    weave_pending(0)
                elif g == 1:
                    weave_pending(1)
                # Weave prerequisite projections into the attention stream
                # so the statically-scheduled PE stream has filler for its
                # idle slices: batch 0/ib 0 pulls in its own k blocks just
                # before the score groups that need them (+ v^T blocks);
                # batch 1's projections ride inside batch 0's later
                # iterations. (qk_half, vT, v_tr) per slot, see table.
                for kind, arg in WEAVE.get((b, ib, g), ()):
                    if kind == "qk":
                        emit_qk_nb(*arg)
                    elif kind == "q":
                        emit_q_nb(*arg)
                    elif kind == "k":
                        emit_k_nb(*arg)
                    elif kind == "v":
                        emit_v(*arg)
                    elif kind == "vT":
                        emit_vT_nb(*arg)
                scA = ps_big.tile([128, 1024], F32, name="scA", tag="sc")
                scB = ps_big.tile([128, 1024], F32, name="scB", tag="sc")
                for idx in range(gw):
                    jt = g0 + idx
                    for hsl, sc in ((slice(0, 64), scA), (slice(64, 128), scB)):
                        nc.tensor.matmul(
                            sc[:, idx * 512:(idx + 1) * 512],
                            lhsT=kTs[b][hsl, jt * 128:(jt + 1) * 128],
                            rhs=qTs[b][hsl, icol:icol + 512],
                            start=True,
                            stop=True,
                        )
                for st_exp, sc in ((stA, scA), (stB, scB)):
                    nc.scalar.activation(
                        out=st_exp[:, g0 * 512:(g0 + gw) * 512],
                        in_=sc[:, 0:gw * 512],
                        func=AF.Exp,
                        scale=0.125,
                    )
                # batch 0's v tiles just in time for the attended matmuls
                # below (b1's were prebuilt in the weave)
                if b == 0 and ib == 0:
                    for idx in range(gw):
                        emit_v(b, g0 + idx)
                # attended for head A consumes the PREVIOUS group's exps
                # (one group of slack so the PE never waits on ScalarE)
                if attA_ps is None:
                    attA_ps = ps_att.tile([128, 512], F32, name="attps", tag="att")
                if g0 >= 2:
                    for jt in (g0 - 2, g0 - 1):
                        nc.tensor.matmul(
                            attA_ps[0:128, :],
                            lhsT=vsbs[b][:, jt * VW: jt * VW + 128],
                            rhs=stA[:, jt * 512:(jt + 1) * 512],
                            start=(jt == 0),
                            stop=False,
                        )

            for jt in (JT - 2, JT - 1):
                nc.tensor.matmul(
                    attA_ps[0:128, :],
                    lhsT=vsbs[b][:, jt * VW: jt * VW + 128],
                    rhs=stA[:, jt * 512:(jt + 1) * 512],
                    start=False,
                    stop=(jt == JT - 1),
                )
            normalize(b, ib, 0, attA_ps)
            pend.update(b=b, ib=ib, stB=stB)

        for b in range(B):
            for ib in range(IB):
                emit_attention_ib(b, ib)
        weave_pending(0)
        weave_pending(1)


def build_nc():
    if "nc" in _NC_CACHE:
        return _NC_CACHE["nc"]
    nc = bacc.Bacc("TRN2", debug=False, num_devices=NCORES)
    xT = nc.dram_tensor("xT", [D, BS], BF16, kind="ExternalInput").ap()
    wqk = nc.dram_tensor("wqk", [D, 256], BF16, kind="ExternalInput").ap()
    wv = nc.dram_tensor("wv", [D, 128], BF16, kind="ExternalInput").ap()
    wo = nc.dram_tensor("wo", [128, D], BF16, kind="ExternalInput").ap()
    out = nc.dram_tensor("out", [BS, D], F32, kind="ExternalOutput").ap()
    with tile.TileContext(nc) as tc:
        _emit(tc, xT, wqk, wv, wo, out)
    nc.compile()
    _NC_CACHE["nc"] = nc
    return nc


def make_in_maps(x, qkv_w):
    """Host-side shard + transpose + cast. Returns per-core input dicts
    (without wo/out, added by caller)."""
    bf = ml_dtypes.bfloat16
    xT = np.ascontiguousarray(x.reshape(BS, D).T).astype(bf)
    maps = []
    for c in range(NCORES):
        wA = qkv_w[c * 384: c * 384 + 192]
        wB = qkv_w[c * 384 + 192: c * 384 + 384]
        wq = np.concatenate([wA[0:64], wB[0:64]], 0)        # [128, D]
        wk = np.concatenate([wA[64:128], wB[64:128]], 0)    # [128, D]
        wv_ = np.concatenate([wA[128:192], wB[128:192]], 0)  # [128, D]
        wqk_c = np.ascontiguousarray(
            np.concatenate([wq, wk], 0).T).astype(bf)        # [D, 256]
        wv_c = np.ascontiguousarray(wv_.T).astype(bf)        # [D, 128]
        maps.append({"xT": xT, "wqk": wqk_c, "wv": wv_c})
    return maps


def kernel(x, qkv_w, out_w, out_b, _run_kwargs=None):
    x = np.asarray(x, dtype=np.float32)
    qkv_w = np.asarray(qkv_w, dtype=np.float32)
    out_w = np.asarray(out_w, dtype=np.float32)
    out_b = np.asarray(out_b, dtype=np.float32)
    bf = ml_dtypes.bfloat16

    nc = build_nc()
    in_maps = make_in_maps(x, qkv_w)
    for c in range(NCORES):
        wo_c = np.ascontiguousarray(
            out_w[:, c * 128:(c + 1) * 128].T).astype(bf)    # [128, D]
        in_maps[c]["wo"] = wo_c

    res = run_bass_kernel_spmd(
        nc, in_maps, list(range(NCORES)), **(_run_kwargs or {})
    )
    total = np.zeros((BS, D), np.float32)
    for c in range(NCORES):
        total += np.asarray(res.results[c]["out"])
    total += out_b[None, :]
    out = total.reshape(B, S, D)
    if _run_kwargs:
        kernel.last_result = res
    return out


# revision 43
# speedup vs baseline: 1.0935x; 1.0202x over previous
"""Trainium2 Bass kernel for multi-head attention (B=2, S=2048, D=1024, H=16, HD=64).

Sharding: tensor-parallel over heads. Each of the 8 cores owns 2 heads
(core c -> heads 2c, 2c+1) and computes:
  - q^T, k^T projections for its heads (layout [head_dim*2, seq]); the two
    heads live on partitions 0:64 / 64:128 so their K=64 score matmuls run
    CONCURRENTLY on the PE via row-group tiling
  - v^T projection (N=512 matmuls) + one PE transpose per s-tile into
    [1 | pad63 | v(64)] blocks, so the attended matmul's stationary operand
    carries a ones column: the attended matmul then emits the softmax
    denominators on PSUM partition 0 for free
  - scores^T = K @ Q^T per (batch, head) in [key, query] layout -> exp on
    ScalarE (reading PSUM directly, 2-key-tile groups) with the 1/sqrt(64)
    scale fused into the activation
  - attended^T = [1|pad|V]^T @ exp(scores^T), normalized via
    reciprocal_approx_fast + a GpSimd partition broadcast
  - partial output projection out_c = attended_c @ out_w[:, heads_c]^T
Host sums the 8 partial outputs and adds the bias.

All input transposes are done host-side (only v^T -> v uses the PE's
transpose mode on-chip). Prerequisite projections are hand-woven into the attention emission order
(the Tile schedule is static per engine) and head B's attended matmuls +
normalize + output projection are software-pipelined one iteration later,
under the next iteration's exp calls.
"""

import numpy as np
import ml_dtypes

import concourse.bacc as bacc
import concourse.tile as tile
import concourse.mybir as mybir
from concourse.bass_utils import run_bass_kernel_spmd
from concourse.masks import make_identity

B, S, D = 2, 2048, 1024
H, HD = 16, 64
FEA = H * HD  # 1024
NCORES = 8
BS = B * S  # 4096

DT_TILES = 8      # 1024 contraction dim / 128
JT = 16           # key tiles of 128 per batch
IB = 4            # query blocks of 512 per batch
VW = 256          # v storage width per s-tile: [1|pad63|vA(64) | 1|pad63|vB(64)]

BF16 = mybir.dt.bfloat16
F32 = mybir.dt.float32
AF = mybir.ActivationFunctionType
ALU = mybir.AluOpType

_NC_CACHE = {}


def _emit(tc, xT, wqk, wv, wo, out):
    nc = tc.nc
    with (
        tc.tile_pool(name="consts", bufs=1) as consts,
        tc.tile_pool(name="stp", bufs=3) as stp,
        tc.tile_pool(name="small", bufs=4) as small,
        tc.tile_pool(name="tmpb", bufs=3) as tmpb,
        tc.tile_pool(name="outsb", bufs=4) as outsb,
        tc.tile_pool(name="ps_big", bufs=2, space="PSUM") as ps_big,
        tc.tile_pool(name="ps_att", bufs=2, space="PSUM") as ps_att,
        tc.tile_pool(name="ps_small", bufs=2, space="PSUM") as ps_small,
    ):
        xts = [consts.tile([128, BS], BF16, name=f"xt{i}", tag=f"xt{i}") for i in range(DT_TILES)]
        wqks = [consts.tile([128, 256], BF16, name=f"wqk{i}", tag=f"wqk{i}") for i in range(DT_TILES)]
        wvs = [consts.tile([128, 128], BF16, name=f"wv{i}", tag=f"wv{i}") for i in range(DT_TILES)]
        wos = consts.tile([128, D], BF16, tag="wo")
        # per-batch activations
        qTs = [consts.tile([128, S], BF16, name=f"qT{b}", tag=f"qT{b}") for b in range(B)]
        kTs = [consts.tile([128, S], BF16, name=f"kT{b}", tag=f"kT{b}") for b in range(B)]
        vsbs = [consts.tile([128, 16 * VW], BF16, name=f"v{b}", tag=f"v{b}") for b in range(B)]
        attTs = [consts.tile([128, S], BF16, name=f"attT{b}", tag=f"attT{b}") for b in range(B)]
        vTs = [consts.tile([128, S], BF16, name=f"vT{b}", tag=f"vT{b}") for b in range(B)]
        ident = consts.tile([128, 128], BF16, tag="ident")

        # load x^T in column chunks so QKV (and then attention) can start
        # before the whole 8MB lands; weights ride along with chunk 0
        for cb in range(8):
            for dt in range(DT_TILES):
                if cb == 0:
                    nc.sync.dma_start(out=wqks[dt], in_=wqk[dt * 128:(dt + 1) * 128, :])
                nc.sync.dma_start(
                    out=xts[dt][:, cb * 512:(cb + 1) * 512],
                    in_=xT[dt * 128:(dt + 1) * 128, cb * 512:(cb + 1) * 512],
                )
                if cb == 0:
                    nc.sync.dma_start(out=wvs[dt], in_=wv[dt * 128:(dt + 1) * 128, :])
        nc.sync.dma_start(out=wos, in_=wo[:, :])
        for b in range(B):
            nc.vector.memset(vsbs[b], 1.0)  # presets the ones columns
        make_identity(nc, ident)

        def _emit_proj_nb(b, nb, half, dst):
            # one 512-col block of a q^T / k^T projection
            scol = nb * 512
            ps = ps_small.tile([128, 512], F32, name="pss", tag="ps_small")
            for dt in range(DT_TILES):
                nc.tensor.matmul(
                    ps,
                    lhsT=wqks[dt][:, half * 128:(half + 1) * 128],
                    rhs=xts[dt][:, b * S + scol: b * S + scol + 512],
                    start=(dt == 0),
                    stop=(dt == DT_TILES - 1),
                )
            nc.vector.tensor_copy(out=dst[:, scol:scol + 512], in_=ps)

        def emit_q_nb(b, nb):
            _emit_proj_nb(b, nb, 0, qTs[b])

        def emit_k_nb(b, nb):
            _emit_proj_nb(b, nb, 1, kTs[b])

        def emit_qk_nb(b, nb):
            emit_k_nb(b, nb)
            emit_q_nb(b, nb)

        def emit_vT_nb(b, nb):
            # v^T [f=128, s] projection block: N=512 matmuls with a reused
            # stationary (much cheaper on the PE than 4 tiny N=128 groups)
            scol = nb * 512
            ps = ps_small.tile([128, 512], F32, name="pss", tag="ps_small")
            for dt in range(DT_TILES):
                nc.tensor.matmul(
                    ps,
                    lhsT=wvs[dt],
                    rhs=xts[dt][:, b * S + scol: b * S + scol + 512],
                    start=(dt == 0),
                    stop=(dt == DT_TILES - 1),
                )
            nc.vector.tensor_copy(out=vTs[b][:, scol:scol + 512], in_=ps)

        def emit_v(b, st):
            # One PE transpose turns v^T's [f=128, s-tile] block into natural
            # [s, f] order, then a strided DVE copy lands it as
            # [1|pad63|vA(64) | 1|pad63|vB(64)]. Ones column first =>
            # softmax denominators land on PSUM partition 0
            # (partition_broadcast needs a partition-0 source); attended rows
            # occupy partitions 64:128 (spans >32 partitions must start at 0
            # or 64). Pad columns are 1.0 -> harmless duplicate denominator
            # rows in PSUM.
            ps = ps_small.tile([128, 128], BF16, name="pss", tag="ps_small")
            nc.tensor.transpose(
                ps, vTs[b][:, st * 128:(st + 1) * 128], ident
            )
            vsrc = ps.rearrange("p (two c) -> p two c", two=2)
            vdst = vsbs[b][:, st * VW: st * VW + VW].rearrange(
                "p (two c) -> p two c", two=2
            )[:, :, 64:128]
            nc.vector.tensor_copy(out=vdst, in_=vsrc)

        def emit_outproj_ib(b, ib):
            # output rows [b*S + ib*512, +512) only need attT cols of this ib
            for st in range(b * 16 + ib * 4, b * 16 + ib * 4 + 4):
                for db in range(2):
                    ps = ps_small.tile([128, 512], F32, name="pss", tag="ps_small")
                    nc.tensor.matmul(
                        ps,
                        lhsT=attTs[b][:, (st - b * 16) * 128:(st - b * 16 + 1) * 128],
                        rhs=wos[:, db * 512:(db + 1) * 512],
                        start=True,
                        stop=True,
                    )
                    osb = outsb.tile([128, 512], F32, name="osb", tag="osb")
                    nc.vector.tensor_copy(out=osb, in_=ps)
                    nc.sync.dma_start(
                        out=out[st * 128:(st + 1) * 128, db * 512:(db + 1) * 512],
                        in_=osb,
                    )

        def normalize(b, ib, h, att_ps):
            icol = ib * 512
            # Drain PSUM immediately (one copy) so the slot frees fast;
            # normalize from the SBUF copy. Row 0 = denominators,
            # rows 64:128 = attended^T.
            araw = small.tile([128, 512], F32, name="araw", tag="araw")
            nc.vector.tensor_copy(out=araw, in_=att_ps[0:128, :])
            rrow = small.tile([1, 512], F32, name="rrow", tag="rrow")
            nc.vector.reciprocal_approx_fast(out=rrow, in_=araw[0:1, :])
            rb = small.tile([128, 512], F32, name="rb", tag="rb")
            nc.gpsimd.partition_broadcast(rb, rrow)
            # tensor_tensor needs both SBUF inputs at the same base
            # partition -> use the 64:128 half of the broadcast.
            if h == 0:
                nc.vector.tensor_tensor(
                    out=attTs[b][0:64, icol:icol + 512],
                    in0=araw[64:128, :],
                    in1=rb[64:128, :],
                    op=ALU.mult,
                )
            else:
                tb = tmpb.tile([64, 512], BF16, name="tb", tag="tb")
                nc.vector.tensor_tensor(
                    out=tb,
                    in0=araw[64:128, :],
                    in1=rb[64:128, :],
                    op=ALU.mult,
                )
                # head B lives at partitions 64:128 of attT; engines
                # can't cross partitions, DMA can.
                nc.sync.dma_start(
                    out=attTs[b][64:128, icol:icol + 512], in_=tb
                )

        # (b, ib, slot) -> prerequisite projection blocks to emit there.
        # k blocks must precede the score groups that read them; vT blocks
        # must precede the v transposes of the s-tiles they cover.
        WEAVE = {
            (0, 0, 0): (("qk", (0, 0)), ("vT", (0, 0))),
            (0, 0, 2): (("k", (0, 1)), ("vT", (0, 1))),
            (0, 0, 3): (("q", (0, 1)),),
            (0, 0, 4): (("k", (0, 2)), ("vT", (0, 2))),
            (0, 0, 5): (("q", (0, 2)),),
            (0, 0, 6): (("k", (0, 3)), ("vT", (0, 3))),
            (0, 0, 7): (("q", (0, 3)),),
            (0, 1, 0): (("k", (1, 0)),),
            (0, 1, 1): (("q", (1, 0)),),
            (0, 1, 2): (("k", (1, 1)),),
            (0, 1, 3): (("q", (1, 1)),),
            (0, 1, 4): (("k", (1, 2)),),
            (0, 1, 5): (("q", (1, 2)),),
            (0, 1, 6): (("k", (1, 3)),),
            (0, 1, 7): (("q", (1, 3)),),
            (0, 2, 0): (("vT", (1, 0)),),
            (0, 2, 1): (("vT", (1, 1)),),
            (0, 2, 2): (("vT", (1, 2)), ("v", (1, 0)), ("v", (1, 1))),
            (0, 2, 3): (("vT", (1, 3)), ("v", (1, 2)), ("v", (1, 3))),
            (0, 2, 4): (("v", (1, 4)), ("v", (1, 5))),
            (0, 2, 5): (("v", (1, 6)), ("v", (1, 7))),
            (0, 2, 6): (("v", (1, 8)), ("v", (1, 9))),
            (0, 2, 7): (("v", (1, 10)), ("v", (1, 11))),
            (0, 3, 0): (("v", (1, 12)), ("v", (1, 13))),
            (0, 3, 1): (("v", (1, 14)), ("v", (1, 15))),
        }

        # Head B's attended matmuls + normalize + outproj of iteration (b,ib)
        # are software-pipelined into the START of the next iteration, where
        # the ScalarE stream still lags the PE (so the PE burst hides under
        # the previous iteration's exp calls).
        pend = {}

        def weave_pending(stage):
            if "b" not in pend:
                return
            pb, pib, pstB = pend["b"], pend["ib"], pend["stB"]
            if stage == 0:
                attB_ps = ps_att.tile([128, 512], F32, name="attps", tag="att")
                pend["ps"] = attB_ps
                jts = range(JT // 2)
            else:
                attB_ps = pend["ps"]
                jts = range(JT // 2, JT)
            for jt in jts:
                nc.tensor.matmul(
                    attB_ps[0:128, :],
                    lhsT=vsbs[pb][:, jt * VW + 128: jt * VW + 256],
                    rhs=pstB[:, jt * 512:(jt + 1) * 512],
                    start=(jt == 0),
                    stop=(jt == JT - 1),
                )
            if stage == 1:
                normalize(pb, pib, 1, pend["ps"])
                emit_outproj_ib(pb, pib)
                pend.clear()

        def emit_attention_ib(b, ib):
            icol = ib * 512
            # Both heads processed together: head A (partitions 0:64) and
            # head B (64:128) score matmuls are emitted interleaved so the
            # PE runs them CONCURRENTLY via row-group tiling (K=64 each ->
            # disjoint row halves of the array).
            stA = stp.tile([128, JT * 512], BF16, name="stA", tag="st")
            stB = stp.tile([128, JT * 512], BF16, name="stB", tag="st")
            attA_ps = None
            for g0 in range(0, JT, 2):
                g = g0 // 2
                gw = min(2, JT - g0)
                if g == 0:
                    weave_pending(0)
                elif g == 1:
                    weave_pending(1)
                # Weave prerequisite projections into the attention stream
                # so the statically-scheduled PE stream has filler for its
                # idle slices: batch 0/ib 0 pulls in its own k blocks just
                # before the score groups that need them (+ v^T blocks);
                # batch 1's projections ride inside batch 0's later
                # iterations. (qk_half, vT, v_tr) per slot, see table.
                for kind, arg in WEAVE.get((b, ib, g), ()):
                    if kind == "qk":
                        emit_qk_nb(*arg)
                    elif kind == "q":
                        emit_q_nb(*arg)
                    elif kind == "k":
                        emit_k_nb(*arg)
                    elif kind == "v":
                        emit_v(*arg)
                    elif kind == "vT":
                        emit_vT_nb(*arg)
                scA = ps_big.tile([128, 1024], F32, name="scA", tag="sc")
                scB = ps_big.tile([128, 1024], F32, name="scB", tag="sc")
                for idx in range(gw):
                    jt = g0 + idx
                    for hsl, sc in ((slice(0, 64), scA), (slice(64, 128), scB)):
                        nc.tensor.matmul(
                            sc[:, idx * 512:(idx + 1) * 512],
                            lhsT=kTs[b][hsl, jt * 128:(jt + 1) * 128],
                            rhs=qTs[b][hsl, icol:icol + 512],
                            start=True,
                            stop=True,
                        )
                for st_exp, sc in ((stA, scA), (stB, scB)):
                    nc.scalar.activation(
                        out=st_exp[:, g0 * 512:(g0 + gw) * 512],
                        in_=sc[:, 0:gw * 512],
                        func=AF.Exp,
                        scale=0.125,
                    )
                # batch 0's v tiles just in time for the attended matmuls
                # below (b1's were prebuilt in the weave)
                if b == 0 and ib == 0:
                    for idx in range(gw):
                        emit_v(b, g0 + idx)
                # attended for head A consumes the PREVIOUS group's exps
                # (one group of slack so the PE never waits on ScalarE)
                if attA_ps is None:
                    attA_ps = ps_att.tile([128, 512], F32, name="attps", tag="att")
                if g0 >= 2:
                    for jt in (g0 - 2, g0 - 1):
                        nc.tensor.matmul(
                            attA_ps[0:128, :],
                            lhsT=vsbs[b][:, jt * VW: jt * VW + 128],
                            rhs=stA[:, jt * 512:(jt + 1) * 512],
                            start=(jt == 0),
                            stop=False,
                        )

            for jt in (JT - 2, JT - 1):
                nc.tensor.matmul(
                    attA_ps[0:128, :],
                    lhsT=vsbs[b][:, jt * VW: jt * VW + 128],
                    rhs=stA[:, jt * 512:(jt + 1) * 512],
                    start=False,
                    stop=(jt == JT - 1),
                )
            normalize(b, ib, 0, attA_ps)
            pend.update(b=b, ib=ib, stB=stB)

        for b in range(B):
            for ib in range(IB):
                emit_attention_ib(b, ib)
        weave_pending(0)
        weave_pending(1)


def build_nc():
    if "nc" in _NC_CACHE:
        return _NC_CACHE["nc"]
    nc = bacc.Bacc("TRN2", debug=False, num_devices=NCORES)
    xT = nc.dram_tensor("xT", [D, BS], BF16, kind="ExternalInput").ap()
    wqk = nc.dram_tensor("wqk", [D, 256], BF16, kind="ExternalInput").ap()
    wv = nc.dram_tensor("wv", [D, 128], BF16, kind="ExternalInput").ap()
    wo = nc.dram_tensor("wo", [128, D], BF16, kind="ExternalInput").ap()
    out = nc.dram_tensor("out", [BS, D], F32, kind="ExternalOutput").ap()
    with tile.TileContext(nc) as tc:
        _emit(tc, xT, wqk, wv, wo, out)
    nc.compile()
    _NC_CACHE["nc"] = nc
    return nc


def make_in_maps(x, qkv_w):
    """Host-side shard + transpose + cast. Returns per-core input dicts
    (without wo/out, added by caller)."""
    bf = ml_dtypes.bfloat16
    xT = np.ascontiguousarray(x.reshape(BS, D).T).astype(bf)
    maps = []
    for c in range(NCORES):
        wA = qkv_w[c * 384: c * 384 + 192]
        wB = qkv_w[c * 384 + 192: c * 384 + 384]
        wq = np.concatenate([wA[0:64], wB[0:64]], 0)        # [128, D]
        wk = np.concatenate([wA[64:128], wB[64:128]], 0)    # [128, D]
        wv_ = np.concatenate([wA[128:192], wB[128:192]], 0)  # [128, D]
        wqk_c = np.ascontiguousarray(
            np.concatenate([wq, wk], 0).T).astype(bf)        # [D, 256]
        wv_c = np.ascontiguousarray(wv_.T).astype(bf)        # [D, 128]
        maps.append({"xT": xT, "wqk": wqk_c, "wv": wv_c})
    return maps


def kernel(x, qkv_w, out_w, out_b, _run_kwargs=None):
    x = np.asarray(x, dtype=np.float32)
    qkv_w = np.asarray(qkv_w, dtype=np.float32)
    out_w = np.asarray(out_w, dtype=np.float32)
    out_b = np.asarray(out_b, dtype=np.float32)
    bf = ml_dtypes.bfloat16

    nc = build_nc()
    in_maps = make_in_maps(x, qkv_w)
    for c in range(NCORES):
        wo_c = np.ascontiguousarray(
            out_w[:, c * 128:(c + 1) * 128].T).astype(bf)    # [128, D]
        in_maps[c]["wo"] = wo_c

    res = run_bass_kernel_spmd(
        nc, in_maps, list(range(NCORES)), **(_run_kwargs or {})
    )
    total = np.zeros((BS, D), np.float32)
    for c in range(NCORES):
        total += np.asarray(res.results[c]["out"])
    total += out_b[None, :]
    out = total.reshape(B, S, D)
    if _run_kwargs:
        kernel.last_result = res
    return out


# revision 44
# speedup vs baseline: 1.0962x; 1.0025x over previous
"""Trainium2 Bass kernel for multi-head attention (B=2, S=2048, D=1024, H=16, HD=64).

Sharding: tensor-parallel over heads. Each of the 8 cores owns 2 heads
(core c -> heads 2c, 2c+1) and computes:
  - q^T, k^T projections for its heads (layout [head_dim*2, seq]); the two
    heads live on partitions 0:64 / 64:128 so their K=64 score matmuls run
    CONCURRENTLY on the PE via row-group tiling
  - v^T projection (N=512 matmuls) + one PE transpose per s-tile into
    [1 | pad63 | v(64)] blocks, so the attended matmul's stationary operand
    carries a ones column: the attended matmul then emits the softmax
    denominators on PSUM partition 0 for free
  - scores^T = K @ Q^T per (batch, head) in [key, query] layout -> exp on
    ScalarE (reading PSUM directly, 2-key-tile groups) with the 1/sqrt(64)
    scale fused into the activation
  - attended^T = [1|pad|V]^T @ exp(scores^T), normalized via
    reciprocal_approx_fast + a GpSimd partition broadcast
  - partial output projection out_c = attended_c @ out_w[:, heads_c]^T
Host sums the 8 partial outputs and adds the bias.

All input transposes are done host-side (only v^T -> v uses the PE's
transpose mode on-chip). Prerequisite projections are hand-woven into the attention emission order
(the Tile schedule is static per engine) and head B's attended matmuls +
normalize + output projection are software-pipelined one iteration later,
under the next iteration's exp calls.
"""

import numpy as np
import ml_dtypes

import concourse.bacc as bacc
import concourse.tile as tile
import concourse.mybir as mybir
from concourse.bass_utils import run_bass_kernel_spmd
from concourse.masks import make_identity

B, S, D = 2, 2048, 1024
H, HD = 16, 64
FEA = H * HD  # 1024
NCORES = 8
BS = B * S  # 4096

DT_TILES = 8      # 1024 contraction dim / 128
JT = 16           # key tiles of 128 per batch
IB = 4            # query blocks of 512 per batch
VW = 256          # v storage width per s-tile: [1|pad63|vA(64) | 1|pad63|vB(64)]

BF16 = mybir.dt.bfloat16
F32 = mybir.dt.float32
AF = mybir.ActivationFunctionType
ALU = mybir.AluOpType

_NC_CACHE = {}


def _emit(tc, xT, wqk, wv, wo, out):
    nc = tc.nc
    with (
        tc.tile_pool(name="consts", bufs=1) as consts,
        tc.tile_pool(name="stp", bufs=3) as stp,
        tc.tile_pool(name="small", bufs=4) as small,
        tc.tile_pool(name="tmpb", bufs=3) as tmpb,
        tc.tile_pool(name="outsb", bufs=4) as outsb,
        tc.tile_pool(name="ps_big", bufs=2, space="PSUM") as ps_big,
        tc.tile_pool(name="ps_att", bufs=2, space="PSUM") as ps_att,
        tc.tile_pool(name="ps_small", bufs=2, space="PSUM") as ps_small,
    ):
        xts = [consts.tile([128, BS], BF16, name=f"xt{i}", tag=f"xt{i}") for i in range(DT_TILES)]
        wqks = [consts.tile([128, 256], BF16, name=f"wqk{i}", tag=f"wqk{i}") for i in range(DT_TILES)]
        wvs = [consts.tile([128, 128], BF16, name=f"wv{i}", tag=f"wv{i}") for i in range(DT_TILES)]
        wos = consts.tile([128, D], BF16, tag="wo")
        # per-batch activations
        qTs = [consts.tile([128, S], BF16, name=f"qT{b}", tag=f"qT{b}") for b in range(B)]
        kTs = [consts.tile([128, S], BF16, name=f"kT{b}", tag=f"kT{b}") for b in range(B)]
        vsbs = [consts.tile([128, 16 * VW], BF16, name=f"v{b}", tag=f"v{b}") for b in range(B)]
        attTs = [consts.tile([128, S], BF16, name=f"attT{b}", tag=f"attT{b}") for b in range(B)]
        vTs = [consts.tile([128, S], BF16, name=f"vT{b}", tag=f"vT{b}") for b in range(B)]
        ident = consts.tile([128, 128], BF16, tag="ident")

        # load x^T in column chunks so QKV (and then attention) can start
        # before the whole 8MB lands; weights ride along with chunk 0
        for cb in range(8):
            for dt in range(DT_TILES):
                if cb == 0:
                    nc.sync.dma_start(out=wqks[dt], in_=wqk[dt * 128:(dt + 1) * 128, :])
                nc.sync.dma_start(
                    out=xts[dt][:, cb * 512:(cb + 1) * 512],
                    in_=xT[dt * 128:(dt + 1) * 128, cb * 512:(cb + 1) * 512],
                )
                if cb == 0:
                    nc.sync.dma_start(out=wvs[dt], in_=wv[dt * 128:(dt + 1) * 128, :])
        nc.sync.dma_start(out=wos, in_=wo[:, :])
        for b in range(B):
            nc.vector.memset(vsbs[b], 1.0)  # presets the ones columns
        make_identity(nc, ident)

        def _emit_proj_nb(b, nb, half, dst):
            # one 512-col block of a q^T / k^T projection
            scol = nb * 512
            ps = ps_small.tile([128, 512], F32, name="pss", tag="ps_small")
            for dt in range(DT_TILES):
                nc.tensor.matmul(
                    ps,
                    lhsT=wqks[dt][:, half * 128:(half + 1) * 128],
                    rhs=xts[dt][:, b * S + scol: b * S + scol + 512],
                    start=(dt == 0),
                    stop=(dt == DT_TILES - 1),
                )
            nc.vector.tensor_copy(out=dst[:, scol:scol + 512], in_=ps)

        def emit_q_nb(b, nb):
            _emit_proj_nb(b, nb, 0, qTs[b])

        def emit_k_nb(b, nb):
            _emit_proj_nb(b, nb, 1, kTs[b])

        def emit_qk_nb(b, nb):
            emit_k_nb(b, nb)
            emit_q_nb(b, nb)

        def emit_vT_nb(b, nb):
            # v^T [f=128, s] projection block: N=512 matmuls with a reused
            # stationary (much cheaper on the PE than 4 tiny N=128 groups)
            scol = nb * 512
            ps = ps_small.tile([128, 512], F32, name="pss", tag="ps_small")
            for dt in range(DT_TILES):
                nc.tensor.matmul(
                    ps,
                    lhsT=wvs[dt],
                    rhs=xts[dt][:, b * S + scol: b * S + scol + 512],
                    start=(dt == 0),
                    stop=(dt == DT_TILES - 1),
                )
            nc.vector.tensor_copy(out=vTs[b][:, scol:scol + 512], in_=ps)

        def emit_v(b, st):
            # One PE transpose turns v^T's [f=128, s-tile] block into natural
            # [s, f] order, then a strided DVE copy lands it as
            # [1|pad63|vA(64) | 1|pad63|vB(64)]. Ones column first =>
            # softmax denominators land on PSUM partition 0
            # (partition_broadcast needs a partition-0 source); attended rows
            # occupy partitions 64:128 (spans >32 partitions must start at 0
            # or 64). Pad columns are 1.0 -> harmless duplicate denominator
            # rows in PSUM.
            ps = ps_small.tile([128, 128], BF16, name="pss", tag="ps_small")
            nc.tensor.transpose(
                ps, vTs[b][:, st * 128:(st + 1) * 128], ident
            )
            vsrc = ps.rearrange("p (two c) -> p two c", two=2)
            vdst = vsbs[b][:, st * VW: st * VW + VW].rearrange(
                "p (two c) -> p two c", two=2
            )[:, :, 64:128]
            nc.vector.tensor_copy(out=vdst, in_=vsrc)

        def emit_outproj_ib(b, ib):
            # output rows [b*S + ib*512, +512) only need attT cols of this ib
            for st in range(b * 16 + ib * 4, b * 16 + ib * 4 + 4):
                for db in range(2):
                    ps = ps_small.tile([128, 512], F32, name="pss", tag="ps_small")
                    nc.tensor.matmul(
                        ps,
                        lhsT=attTs[b][:, (st - b * 16) * 128:(st - b * 16 + 1) * 128],
                        rhs=wos[:, db * 512:(db + 1) * 512],
                        start=True,
                        stop=True,
                    )
                    osb = outsb.tile([128, 512], F32, name="osb", tag="osb")
                    nc.vector.tensor_copy(out=osb, in_=ps)
                    nc.sync.dma_start(
                        out=out[st * 128:(st + 1) * 128, db * 512:(db + 1) * 512],
                        in_=osb,
                    )

        def normalize(b, ib, h, att_ps):
            icol = ib * 512
            # Drain PSUM immediately (one copy) so the slot frees fast;
            # normalize from the SBUF copy. Row 0 = denominators,
            # rows 64:128 = attended^T.
            araw = small.tile([128, 512], F32, name="araw", tag="araw")
            nc.vector.tensor_copy(out=araw, in_=att_ps[0:128, :])
            rrow = small.tile([1, 512], F32, name="rrow", tag="rrow")
            nc.vector.reciprocal_approx_fast(out=rrow, in_=araw[0:1, :])
            rb = small.tile([128, 512], F32, name="rb", tag="rb")
            nc.gpsimd.partition_broadcast(rb, rrow)
            # tensor_tensor needs both SBUF inputs at the same base
            # partition -> use the 64:128 half of the broadcast.
            if h == 0:
                nc.vector.tensor_tensor(
                    out=attTs[b][0:64, icol:icol + 512],
                    in0=araw[64:128, :],
                    in1=rb[64:128, :],
                    op=ALU.mult,
                )
            else:
                tb = tmpb.tile([64, 512], BF16, name="tb", tag="tb")
                nc.vector.tensor_tensor(
                    out=tb,
                    in0=araw[64:128, :],
                    in1=rb[64:128, :],
                    op=ALU.mult,
                )
                # head B lives at partitions 64:128 of attT; engines
                # can't cross partitions, DMA can.
                nc.sync.dma_start(
                    out=attTs[b][64:128, icol:icol + 512], in_=tb
                )

        # (b, ib, slot) -> prerequisite projection blocks to emit there.
        # k blocks must precede the score groups that read them; vT blocks
        # must precede the v transposes of the s-tiles they cover.
        WEAVE = {
            (0, 0, 0): (("qk", (0, 0)), ("vT", (0, 0))),
            (0, 0, 2): (("k", (0, 1)), ("vT", (0, 1))),
            (0, 0, 3): (("q", (0, 1)),),
            (0, 0, 4): (("k", (0, 2)), ("vT", (0, 2))),
            (0, 0, 5): (("q", (0, 2)),),
            (0, 0, 6): (("k", (0, 3)), ("vT", (0, 3))),
            (0, 0, 7): (("q", (0, 3)),),
            (0, 1, 0): (("k", (1, 0)),),
            (0, 1, 1): (("q", (1, 0)),),
            (0, 1, 2): (("k", (1, 1)),),
            (0, 1, 3): (("q", (1, 1)),),
            (0, 1, 4): (("k", (1, 2)),),
            (0, 1, 5): (("q", (1, 2)),),
            (0, 1, 6): (("k", (1, 3)),),
            (0, 1, 7): (("q", (1, 3)),),
            (0, 2, 0): (("vT", (1, 0)),),
            (0, 2, 1): (("vT", (1, 1)),),
            (0, 2, 2): (("vT", (1, 2)), ("v", (1, 0)), ("v", (1, 1))),
            (0, 2, 3): (("vT", (1, 3)), ("v", (1, 2)), ("v", (1, 3))),
            (0, 2, 4): (("v", (1, 4)), ("v", (1, 5))),
            (0, 2, 5): (("v", (1, 6)), ("v", (1, 7))),
            (0, 2, 6): (("v", (1, 8)), ("v", (1, 9))),
            (0, 2, 7): (("v", (1, 10)), ("v", (1, 11))),
            (0, 3, 0): (("v", (1, 12)), ("v", (1, 13))),
            (0, 3, 1): (("v", (1, 14)), ("v", (1, 15))),
        }

        # Head B's attended matmuls + normalize + outproj of iteration (b,ib)
        # are software-pipelined into the START of the next iteration, where
        # the ScalarE stream still lags the PE (so the PE burst hides under
        # the previous iteration's exp calls).
        pend = {}

        def weave_pending(stage):
            if "b" not in pend:
                return
            pb, pib, pstB = pend["b"], pend["ib"], pend["stB"]
            if stage == 0:
                attB_ps = ps_att.tile([128, 512], F32, name="attps", tag="att")
                pend["ps"] = attB_ps
            else:
                attB_ps = pend["ps"]
            q = JT // 4
            for jt in range(stage * q, (stage + 1) * q):
                nc.tensor.matmul(
                    attB_ps[0:128, :],
                    lhsT=vsbs[pb][:, jt * VW + 128: jt * VW + 256],
                    rhs=pstB[:, jt * 512:(jt + 1) * 512],
                    start=(jt == 0),
                    stop=(jt == JT - 1),
                )
            if stage == 3:
                normalize(pb, pib, 1, pend["ps"])
                emit_outproj_ib(pb, pib)
                pend.clear()

        def emit_attention_ib(b, ib):
            icol = ib * 512
            # Both heads processed together: head A (partitions 0:64) and
            # head B (64:128) score matmuls are emitted interleaved so the
            # PE runs them CONCURRENTLY via row-group tiling (K=64 each ->
            # disjoint row halves of the array).
            stA = stp.tile([128, JT * 512], BF16, name="stA", tag="st")
            stB = stp.tile([128, JT * 512], BF16, name="stB", tag="st")
            attA_ps = None
            for g0 in range(0, JT, 2):
                g = g0 // 2
                gw = min(2, JT - g0)
                if g < 4:
                    weave_pending(g)
                # Weave prerequisite projections into the attention stream
                # so the statically-scheduled PE stream has filler for its
                # idle slices: batch 0/ib 0 pulls in its own k blocks just
                # before the score groups that need them (+ v^T blocks);
                # batch 1's projections ride inside batch 0's later
                # iterations. (qk_half, vT, v_tr) per slot, see table.
                for kind, arg in WEAVE.get((b, ib, g), ()):
                    if kind == "qk":
                        emit_qk_nb(*arg)
                    elif kind == "q":
                        emit_q_nb(*arg)
                    elif kind == "k":
                        emit_k_nb(*arg)
                    elif kind == "v":
                        emit_v(*arg)
                    elif kind == "vT":
                        emit_vT_nb(*arg)
                scA = ps_big.tile([128, 1024], F32, name="scA", tag="sc")
                scB = ps_big.tile([128, 1024], F32, name="scB", tag="sc")
                for idx in range(gw):
                    jt = g0 + idx
                    for hsl, sc in ((slice(0, 64), scA), (slice(64, 128), scB)):
                        nc.tensor.matmul(
                            sc[:, idx * 512:(idx + 1) * 512],
                            lhsT=kTs[b][hsl, jt * 128:(jt + 1) * 128],
                            rhs=qTs[b][hsl, icol:icol + 512],
                            start=True,
                            stop=True,
                        )
                for st_exp, sc in ((stA, scA), (stB, scB)):
                    nc.scalar.activation(
                        out=st_exp[:, g0 * 512:(g0 + gw) * 512],
                        in_=sc[:, 0:gw * 512],
                        func=AF.Exp,
                        scale=0.125,
                    )
                # batch 0's v tiles just in time for the attended matmuls
                # below (b1's were prebuilt in the weave)
                if b == 0 and ib == 0:
                    for idx in range(gw):
                        emit_v(b, g0 + idx)
                # attended for head A consumes the PREVIOUS group's exps
                # (one group of slack so the PE never waits on ScalarE)
                if attA_ps is None:
                    attA_ps = ps_att.tile([128, 512], F32, name="attps", tag="att")
                if g0 >= 2:
                    for jt in (g0 - 2, g0 - 1):
                        nc.tensor.matmul(
                            attA_ps[0:128, :],
                            lhsT=vsbs[b][:, jt * VW: jt * VW + 128],
                            rhs=stA[:, jt * 512:(jt + 1) * 512],
                            start=(jt == 0),
                            stop=False,
                        )

            for jt in (JT - 2, JT - 1):
                nc.tensor.matmul(
                    attA_ps[0:128, :],
                    lhsT=vsbs[b][:, jt * VW: jt * VW + 128],
                    rhs=stA[:, jt * 512:(jt + 1) * 512],
                    start=False,
                    stop=(jt == JT - 1),
                )
            normalize(b, ib, 0, attA_ps)
            pend.update(b=b, ib=ib, stB=stB)

        for b in range(B):
            for ib in range(IB):
                emit_attention_ib(b, ib)
        for _stage in range(4):
            weave_pending(_stage)


def build_nc():
    if "nc" in _NC_CACHE:
        return _NC_CACHE["nc"]
    nc = bacc.Bacc("TRN2", debug=False, num_devices=NCORES)
    xT = nc.dram_tensor("xT", [D, BS], BF16, kind="ExternalInput").ap()
    wqk = nc.dram_tensor("wqk", [D, 256], BF16, kind="ExternalInput").ap()
    wv = nc.dram_tensor("wv", [D, 128], BF16, kind="ExternalInput").ap()
    wo = nc.dram_tensor("wo", [128, D], BF16, kind="ExternalInput").ap()
    out = nc.dram_tensor("out", [BS, D], F32, kind="ExternalOutput").ap()
    with tile.TileContext(nc) as tc:
        _emit(tc, xT, wqk, wv, wo, out)
    nc.compile()
    _NC_CACHE["nc"] = nc
    return nc


def make_in_maps(x, qkv_w):
    """Host-side shard + transpose + cast. Returns per-core input dicts
    (without wo/out, added by caller)."""
    bf = ml_dtypes.bfloat16
    xT = np.ascontiguousarray(x.reshape(BS, D).T).astype(bf)
    maps = []
    for c in range(NCORES):
        wA = qkv_w[c * 384: c * 384 + 192]
        wB = qkv_w[c * 384 + 192: c * 384 + 384]
        wq = np.concatenate([wA[0:64], wB[0:64]], 0)        # [128, D]
        wk = np.concatenate([wA[64:128], wB[64:128]], 0)    # [128, D]
        wv_ = np.concatenate([wA[128:192], wB[128:192]], 0)  # [128, D]
        wqk_c = np.ascontiguousarray(
            np.concatenate([wq, wk], 0).T).astype(bf)        # [D, 256]
        wv_c = np.ascontiguousarray(wv_.T).astype(bf)        # [D, 128]
        maps.append({"xT": xT, "wqk": wqk_c, "wv": wv_c})
    return maps


def kernel(x, qkv_w, out_w, out_b, _run_kwargs=None):
    x = np.asarray(x, dtype=np.float32)
    qkv_w = np.asarray(qkv_w, dtype=np.float32)
    out_w = np.asarray(out_w, dtype=np.float32)
    out_b = np.asarray(out_b, dtype=np.float32)
    bf = ml_dtypes.bfloat16

    nc = build_nc()
    in_maps = make_in_maps(x, qkv_w)
    for c in range(NCORES):
        wo_c = np.ascontiguousarray(
            out_w[:, c * 128:(c + 1) * 128].T).astype(bf)    # [128, D]
        in_maps[c]["wo"] = wo_c

    res = run_bass_kernel_spmd(
        nc, in_maps, list(range(NCORES)), **(_run_kwargs or {})
    )
    total = np.zeros((BS, D), np.float32)
    for c in range(NCORES):
        total += np.asarray(res.results[c]["out"])
    total += out_b[None, :]
    out = total.reshape(B, S, D)
    if _run_kwargs:
        kernel.last_result = res
    return out
